# revision 6
# baseline (speedup 1.0000x reference)
"""Trainium2 Bass kernel for nn_Aggregator (context attention aggregator).

Reference computation (per batch b, with c=cc=128, hw=6400):
  qk   = w_qk @ feat_ctx                  # [256, hw]
  q    = scale * qk[:128]; k = qk[128:]   # [128, hw]
  attn = softmax_over_m(k.T @ q)          # [m=hw, n=hw]
  v    = w_v @ feat_mo                    # [128, hw]
  out  = feat_mo + gamma * (v @ attn)     # [128, hw]

Sharding: 8 cores, data-parallel over batch (4 cores/batch), each core owns a
1600-column slice of the n (query) axis and computes K/V for its full batch
locally (no collectives).  Flash-style: the hw x hw attention matrix is never
materialized in HBM.
"""

import os
import sys
import types

import numpy as np
import ml_dtypes

import concourse.bass as bass
import concourse.tile as tile
from concourse import bacc, mybir
from concourse.bass_utils import run_bass_kernel_spmd

# ---------------------------------------------------------------------------
# Environment fixes (self-contained on purpose: the grading harness imports
# only this file).
# ---------------------------------------------------------------------------


def _install_axon_profile_hook():
    """The image's `antenv` stub lacks `axon_hooks`; run_bass_kernel_spmd
    imports it when trace=True under axon.  Register a functional stand-in."""
    if "antenv.axon_hooks" in sys.modules:
        return
    mod = types.ModuleType("antenv.axon_hooks")
    _hook = [None]
    mod.set_axon_ntff_profile_hook = lambda h: _hook.__setitem__(0, h)
    mod.get_axon_ntff_profile_hook = lambda: _hook[0]
    sys.modules["antenv.axon_hooks"] = mod
    try:
        import antenv

        antenv.axon_hooks = mod
    except Exception:
        pass
    try:
        from trn_agent_boot.trn_boot import _ntff_profile_via_ctypes

        mod.set_axon_ntff_profile_hook(
            _ntff_profile_via_ctypes("/opt/axon/libaxon_pjrt.so")
        )
    except Exception:
        pass


def _install_tile_drain_patch():
    """walrus in this toolchain rejects >1 sync-wait on one CTRL instruction
    ("Too many sync wait commands").  TileContext's final drain carries one
    wait per live semaphore; split them onto individual SP nops."""
    if getattr(tile.TileContext, "_drain_patch_installed", False):
        return
    from concourse.vector_clock import ScopedClock

    def _patched(self, tick_clock, wait_clock):
        nc = self.nc
        probe = nc.sync.nop()
        wait_clock.add_sem_waits(
            probe.ins, ScopedClock({None: tick_clock.global_clock})
        )
        si = probe.ins.sync_info
        waits = list(si.on_wait) if si and si.on_wait else []
        if len(waits) > 1:
            si.on_wait = waits[:1]
            for w in waits[1:]:
                nw = nc.sync.nop()
                nsi = nw.ins.sync_info
                if nsi is None:
                    nw.ins.sync_info = mybir.SyncInfo(on_wait=[w], on_update=[])
                else:
                    nsi.on_wait = [w]
        nc.sync.drain()
        nc.all_engine_barrier()
        assert self.sems is not None
        popped = nc._tile_sem_poison_stack.pop()
        assert popped is self._sem_poison
        nc.clear_and_free_semaphores(list(self.sems.allocated().values()))
        nc.all_engine_barrier()

    tile.TileContext._drain_and_barrier = _patched
    tile.TileContext._drain_patch_installed = True


_install_axon_profile_hook()
_install_tile_drain_patch()

# ---------------------------------------------------------------------------
# Problem constants (hardcoded per spec)
# ---------------------------------------------------------------------------
B = 2          # batch
C = 128        # channels (both ctx and v)
H = W = 80
HW = H * W     # 6400
NCORES = 8
CORES_PER_B = NCORES // B      # 4
NSLC = HW // CORES_PER_B       # 1600 query columns per core
SCALE = C ** -0.5

MCH = HW // 128                # 50 m-chunks of 128
MGRP = 2                       # m-chunks per exp group
N_TILES = [(0, 512), (512, 512), (1024, 512), (1536, 64)]   # per-core n tiling
K_CHUNKS = [(i * 512, 512) for i in range(12)] + [(6144, 256)]

F32 = mybir.dt.float32
BF16 = mybir.dt.bfloat16

_CACHE = {}


def _build():
    nc = bacc.Bacc("TRN2", target_bir_lowering=False, debug=False,
                   num_devices=NCORES)

    fctx = nc.dram_tensor("fctx", [C, HW], BF16, kind="ExternalInput").ap()
    fmo = nc.dram_tensor("fmo", [C, HW], BF16, kind="ExternalInput").ap()
    fres = nc.dram_tensor("fres", [C, NSLC], F32, kind="ExternalInput").ap()
    wq = nc.dram_tensor("wq", [C, C], BF16, kind="ExternalInput").ap()
    wk = nc.dram_tensor("wk", [C, C], BF16, kind="ExternalInput").ap()
    wv = nc.dram_tensor("wv", [C, C], BF16, kind="ExternalInput").ap()
    out = nc.dram_tensor("out", [C, NSLC], F32, kind="ExternalOutput").ap()

    with tile.TileContext(nc) as tc:
        with (
            tc.tile_pool(name="weights", bufs=1) as wpool,
            tc.tile_pool(name="io", bufs=1) as io,
            tc.tile_pool(name="exps", bufs=3) as exps,
            tc.tile_pool(name="small", bufs=3) as small,
            tc.tile_pool(name="psum_s", bufs=2, space="PSUM") as psum_s,
            tc.tile_pool(name="psum_o", bufs=4, space="PSUM") as psum_o,
        ):
            # ---- load inputs -------------------------------------------------
            wq_sb = wpool.tile([C, C], BF16)
            nc.sync.dma_start(wq_sb[:], wq[:])
            wk_sb = wpool.tile([C, C], BF16)
            nc.sync.dma_start(wk_sb[:], wk[:])
            wv_sb = wpool.tile([C, C], BF16)
            nc.sync.dma_start(wv_sb[:], wv[:])
            ident = wpool.tile([C, C], F32)
            from concourse.masks import make_identity
            make_identity(nc, ident)

            fctx_sb = io.tile([C, HW], BF16)
            nc.sync.dma_start(fctx_sb[:], fctx[:])
            fmo_sb = io.tile([C, HW], BF16)
            nc.sync.dma_start(fmo_sb[:], fmo[:])
            fres_sb = io.tile([C, NSLC], F32)
            nc.sync.dma_start(fres_sb[:], fres[:])

            k_sb = io.tile([C, HW], BF16)
            q_sb = io.tile([C, NSLC], BF16)
            vt_sb = io.tile([C, MCH, 129], BF16)

            # ---- projections -------------------------------------------------
            # K[c, m] = sum_i wk[i, c] * fctx[i, m]
            for off, sz in K_CHUNKS:
                ps = psum_s.tile([128, MGRP * 512], F32, tag="ps")
                nc.tensor.matmul(ps[:, :sz], lhsT=wk_sb[:], rhs=fctx_sb[:, off:off + sz],
                                 start=True, stop=True)
                nc.vector.tensor_copy(out=k_sb[:, off:off + sz], in_=ps[:, :sz])
            # Q[c, n] = sum_i wq[i, c] * fctx[i, n0 + n]   (scale folded into wq)
            n0 = 0  # per-core offset comes via the fctx slice below
            # NOTE: q uses this core's n-slice of fctx; the host passes the
            # full-batch fctx, so slice here with the per-core offset encoded
            # by a separate input?  Simpler: host passes fctx already full and
            # ALSO passes the slice offset implicitly by giving each core the
            # same layout; we read fctx[:, qoff + ...] where qoff differs per
            # core.  SPMD means one program for all cores, so instead the host
            # rotates fctx per core such that its n-slice is always the FIRST
            # NSLC columns.  (K uses all columns; attention is permutation-
            # invariant in m, and the colsum likewise.)
            for off, sz in N_TILES:
                ps = psum_s.tile([128, MGRP * 512], F32, tag="ps")
                nc.tensor.matmul(ps[:, :sz], lhsT=wq_sb[:], rhs=fctx_sb[:, off:off + sz],
                                 start=True, stop=True)
                nc.vector.tensor_copy(out=q_sb[:, off:off + sz], in_=ps[:, :sz])
            # VT[m, c] = sum_i fmo[i, m] * wv[i, c]  (gamma folded into wv)
            for mc in range(MCH):
                ps = psum_s.tile([128, MGRP * 512], F32, tag="ps")
                nc.tensor.matmul(ps[:, :C], lhsT=fmo_sb[:, mc * 128:(mc + 1) * 128],
                                 rhs=wv_sb[:], start=True, stop=True)
                nc.vector.tensor_copy(out=vt_sb[:, mc, 0:C], in_=ps[:, :C])
            nc.vector.memset(vt_sb[:, :, 128], 1.0)

            # ---- attention ---------------------------------------------------
            NG = MCH // MGRP  # 25 exp groups per n-tile
            for nt_off, nt_sz in N_TILES:
                n_subs = (nt_sz + 127) // 128
                po = [psum_o.tile([128, 129], F32, tag="po", name=f"po_{nt_off}_{i}")
                      for i in range(n_subs)]
                prev = None  # (expS tile, group index) pending AV
                for g in range(NG + 1):
                    if g < NG:
                        ps = psum_s.tile([128, MGRP, nt_sz], F32, tag="ps")
                        for h in range(MGRP):
                            mc = g * MGRP + h
                            nc.tensor.matmul(
                                ps[:, h, :],
                                lhsT=k_sb[:, mc * 128:(mc + 1) * 128],
                                rhs=q_sb[:, nt_off:nt_off + nt_sz],
                                start=True, stop=True)
                        es = exps.tile([128, MGRP, nt_sz], BF16, tag="es")
                        nc.scalar.activation(
                            out=es[:], in_=ps[:],
                            func=mybir.ActivationFunctionType.Exp)
                        cur = (es, g)
                    else:
                        cur = None
                    if prev is not None:
                        es_p, gp = prev
                        for h in range(MGRP):
                            mc = gp * MGRP + h
                            for ns in range(n_subs):
                                ns_sz = min(128, nt_sz - ns * 128)
                                nc.tensor.matmul(
                                    po[ns][:ns_sz, :],
                                    lhsT=es_p[:, h, ns * 128:ns * 128 + ns_sz],
                                    rhs=vt_sb[:, mc, :],
                                    start=(mc == 0), stop=(mc == MCH - 1),
                                    skip_group_check=True)
                    prev = cur

                # normalize + transpose + residual + store
                for ns in range(n_subs):
                    ns_sz = min(128, nt_sz - ns * 128)
                    recip = small.tile([128, 1], F32, tag="recip")
                    nc.vector.reciprocal(recip[:ns_sz], po[ns][:ns_sz, 128:129])
                    onorm = small.tile([128, 128], F32, tag="onorm")
                    nc.vector.tensor_scalar_mul(
                        onorm[:ns_sz, :], po[ns][:ns_sz, 0:C], recip[:ns_sz])
                    pt = psum_s.tile([128, MGRP * 512], F32, tag="ps")
                    nc.tensor.transpose(pt[:, :ns_sz], onorm[:ns_sz, :],
                                        ident[:ns_sz, :ns_sz])
                    outt = small.tile([128, 128], F32, tag="outt")
                    nc.vector.tensor_add(
                        out=outt[:, :ns_sz], in0=pt[:, :ns_sz],
                        in1=fres_sb[:, nt_off + ns * 128:nt_off + ns * 128 + ns_sz])
                    nc.sync.dma_start(
                        out[:, nt_off + ns * 128:nt_off + ns * 128 + ns_sz],
                        outt[:, :ns_sz])
    nc.compile()
    return nc


def kernel(feat_ctx, feat_mo, w_qk, w_v, gamma, itr=0, **_unused):
    feat_ctx = np.asarray(feat_ctx, dtype=np.float32).reshape(B, C, HW)
    feat_mo = np.asarray(feat_mo, dtype=np.float32).reshape(B, C, HW)
    w_qk = np.asarray(w_qk, dtype=np.float32)
    w_v = np.asarray(w_v, dtype=np.float32)
    gamma_v = float(np.asarray(gamma).reshape(-1)[0])

    bf = ml_dtypes.bfloat16
    wqT = np.ascontiguousarray((SCALE * w_qk[:C]).T).astype(bf)
    wkT = np.ascontiguousarray(w_qk[C:].T).astype(bf)
    wvT = np.ascontiguousarray((gamma_v * w_v).T).astype(bf)

    fctx_bf = feat_ctx.astype(bf)
    fmo_bf = feat_mo.astype(bf)

    if "nc" not in _CACHE:
        _CACHE["nc"] = _build()
    nc = _CACHE["nc"]

    in_maps = []
    for core in range(NCORES):
        b = core // CORES_PER_B
        s = (core % CORES_PER_B) * NSLC
        # Rotate the columns of fctx so this core's query slice is columns
        # [0, NSLC).  K and the softmax normalizer are sums over all m, which
        # are invariant to this permutation; V must use the SAME permutation
        # as K (attn[m, n] pairs with v[:, m]), and rotating both keeps the
        # pairing intact.
        perm_ctx = np.roll(fctx_bf[b], -s, axis=1)
        perm_mo = np.roll(fmo_bf[b], -s, axis=1)
        in_maps.append({
            "fctx": np.ascontiguousarray(perm_ctx),
            "fmo": np.ascontiguousarray(perm_mo),
            "fres": np.ascontiguousarray(feat_mo[b][:, s:s + NSLC]),
            "wq": wqT, "wk": wkT, "wv": wvT,
        })

    trace = bool(int(os.environ.get("KERNEL_TRACE", "0")))
    res = run_bass_kernel_spmd(nc, in_maps, core_ids=list(range(NCORES)),
                               trace=trace)
    kernel.last_exec_time_ns = res.exec_time_ns

    out = np.empty((B, C, HW), dtype=np.float32)
    for core in range(NCORES):
        b = core // CORES_PER_B
        s = (core % CORES_PER_B) * NSLC
        out[b][:, s:s + NSLC] = res.results[core]["out"]
    return out.reshape(B, C, H, W)


# revision 13
# speedup vs baseline: 1.2808x; 1.2808x over previous
"""Trainium2 Bass kernel for nn_Aggregator (context attention aggregator).

Reference computation (per batch b, with c=128, hw=6400):
  q    = scale * (Wq @ X);  k = Wk @ X          # X = feat_ctx [128, hw]
  attn = softmax_over_m(k.T @ q)                # [m=hw, n=hw]
  out  = feat_mo + gamma * ((Wv @ feat_mo) @ attn)

Algebraic folds (host-side, exact f32):
  S = X.T @ M @ X  with  M = scale * Wk.T @ Wq  -> no K projection on device
  gamma * (Wv @ Fm) @ attn = Wvg @ (Fm @ attn)  -> Wv applied AFTER attention
    (Wvg = gamma * Wv), so the device needs Fm only in transposed [m, c]
    layout, which the host provides directly (with a ones column appended so
    the same accumulation produces the softmax denominator for free).

Sharding: 8 cores, data-parallel over batch (4 cores/batch); each core owns
1600 query columns (the host rotates the hw axis per core so its slice is
always columns [0,1600) -- softmax over m is permutation invariant as long as
K and V use the same permutation).  Flash-style: the hw x hw attention matrix
never leaves PSUM/SBUF tiles.
"""

import os
import sys
import types

import numpy as np
import ml_dtypes

import concourse.bass as bass
import concourse.tile as tile
from concourse import bacc, mybir
from concourse.bass_utils import run_bass_kernel_spmd

# ---------------------------------------------------------------------------
# Environment fixes (self-contained on purpose: the grading harness imports
# only this file).
# ---------------------------------------------------------------------------


def _install_axon_profile_hook():
    """The image's `antenv` stub lacks `axon_hooks`; run_bass_kernel_spmd
    imports it when trace=True under axon.  Register a functional stand-in."""
    if "antenv.axon_hooks" in sys.modules:
        return
    mod = types.ModuleType("antenv.axon_hooks")
    _hook = [None]
    mod.set_axon_ntff_profile_hook = lambda h: _hook.__setitem__(0, h)
    mod.get_axon_ntff_profile_hook = lambda: _hook[0]
    sys.modules["antenv.axon_hooks"] = mod
    try:
        import antenv

        antenv.axon_hooks = mod
    except Exception:
        pass
    try:
        from trn_agent_boot.trn_boot import _ntff_profile_via_ctypes

        mod.set_axon_ntff_profile_hook(
            _ntff_profile_via_ctypes("/opt/axon/libaxon_pjrt.so")
        )
    except Exception:
        pass


def _install_tile_drain_patch():
    """walrus in this toolchain rejects >1 sync-wait on one CTRL instruction
    ("Too many sync wait commands").  TileContext's final drain carries one
    wait per live semaphore; split them onto individual SP nops."""
    if getattr(tile.TileContext, "_drain_patch_installed", False):
        return
    from concourse.vector_clock import ScopedClock

    def _patched(self, tick_clock, wait_clock):
        nc = self.nc
        probe = nc.sync.nop()
        wait_clock.add_sem_waits(
            probe.ins, ScopedClock({None: tick_clock.global_clock})
        )
        si = probe.ins.sync_info
        waits = list(si.on_wait) if si and si.on_wait else []
        if len(waits) > 1:
            si.on_wait = waits[:1]
            for w in waits[1:]:
                nw = nc.sync.nop()
                nsi = nw.ins.sync_info
                if nsi is None:
                    nw.ins.sync_info = mybir.SyncInfo(on_wait=[w], on_update=[])
                else:
                    nsi.on_wait = [w]
        nc.sync.drain()
        nc.all_engine_barrier()
        assert self.sems is not None
        popped = nc._tile_sem_poison_stack.pop()
        assert popped is self._sem_poison
        nc.clear_and_free_semaphores(list(self.sems.allocated().values()))
        nc.all_engine_barrier()

    tile.TileContext._drain_and_barrier = _patched
    tile.TileContext._drain_patch_installed = True


_install_axon_profile_hook()
_install_tile_drain_patch()

# ---------------------------------------------------------------------------
# Problem constants (hardcoded per spec)
# ---------------------------------------------------------------------------
B = 2          # batch
C = 128        # channels
H = W = 80
HW = H * W     # 6400
NCORES = 8
CORES_PER_B = NCORES // B      # 4
NSLC = HW // CORES_PER_B       # 1600 query columns per core
SCALE = C ** -0.5

MCH = HW // 128                # 50 m-chunks of 128
MGRP = 2                       # m-chunks per exp group (bank-aligned at 512)
N_TILES = [(0, 512), (512, 512), (1024, 512), (1536, 64)]

F32 = mybir.dt.float32
BF16 = mybir.dt.bfloat16

_CACHE = {}


def _build():
    nc = bacc.Bacc("TRN2", target_bir_lowering=False, debug=False,
                   num_devices=NCORES)

    fctx = nc.dram_tensor("fctx", [C, HW], BF16, kind="ExternalInput").ap()
    fmt = nc.dram_tensor("fmt", [C, MCH, 129], BF16, kind="ExternalInput").ap()
    fres = nc.dram_tensor("fres", [C, NSLC], F32, kind="ExternalInput").ap()
    mq = nc.dram_tensor("mq", [C, C], BF16, kind="ExternalInput").ap()
    wvg = nc.dram_tensor("wvg", [C, C], BF16, kind="ExternalInput").ap()
    out = nc.dram_tensor("out", [C, NSLC], F32, kind="ExternalOutput").ap()

    with tile.TileContext(nc) as tc:
        with (
            tc.tile_pool(name="weights", bufs=1) as wpool,
            tc.tile_pool(name="io", bufs=1) as io,
            tc.tile_pool(name="exps", bufs=3) as exps,
            tc.tile_pool(name="small", bufs=3) as small,
            tc.tile_pool(name="psum_s", bufs=2, space="PSUM") as psum_s,
            tc.tile_pool(name="psum_o", bufs=1, space="PSUM") as psum_o,
            tc.tile_pool(name="psum_e", bufs=2, space="PSUM") as psum_e,
        ):
            # ---- load inputs (small weights first, big tensors chunked) ----
            mq_sb = wpool.tile([C, C], BF16)
            nc.sync.dma_start(mq_sb[:], mq[:])
            wvg_sb = wpool.tile([C, C], BF16)
            nc.sync.dma_start(wvg_sb[:], wvg[:])

            fctx_sb = io.tile([C, HW], BF16)
            for i in range(4):
                nc.sync.dma_start(fctx_sb[:, i * 1600:(i + 1) * 1600],
                                  fctx[:, i * 1600:(i + 1) * 1600])
            fmt_sb = io.tile([C, MCH, 129], BF16)
            for i in range(5):
                nc.sync.dma_start(fmt_sb[:, i * 10:(i + 1) * 10, :],
                                  fmt[:, i * 10:(i + 1) * 10, :])
            fres_sb = io.tile([C, NSLC], F32)
            nc.sync.dma_start(fres_sb[:], fres[:])

            ident = wpool.tile([C, C], BF16)
            from concourse.masks import make_identity
            make_identity(nc, ident)

            # ---- Q' projection: q'[i, n] = sum_j M[i, j] X[j, n0 + n] -----
            q_sb = io.tile([C, NSLC], BF16)
            for nt_off, nt_sz in N_TILES:
                ps = psum_s.tile([128, MGRP * 512], F32, tag="ps", name="psq")
                nc.tensor.matmul(ps[:, :nt_sz], lhsT=mq_sb[:],
                                 rhs=fctx_sb[:, nt_off:nt_off + nt_sz],
                                 start=True, stop=True)
                nc.vector.tensor_copy(out=q_sb[:, nt_off:nt_off + nt_sz],
                                      in_=ps[:, :nt_sz])

            # ---- attention ------------------------------------------------
            for nt_off, nt_sz in N_TILES:
                n_subs = (nt_sz + 127) // 128
                # accumulators: po[:, ns >> 1, ns & 1, :] is one [*, 129]
                # block; the 2x129 pairs stay inside one 512-f32 PSUM bank.
                po = psum_o.tile([128, (n_subs + 1) // 2, 2, 129], F32,
                                 tag="po", name=f"po_{nt_off}")
                groups = [list(range(g, min(g + MGRP, MCH)))
                          for g in range(0, MCH, MGRP)]
                prev = None  # (expS tile, chunk list) pending AV
                for gi in range(len(groups) + 1):
                    if gi < len(groups):
                        mcs = groups[gi]
                        ng = len(mcs)
                        ps = psum_s.tile([128, MGRP * 512], F32, tag="ps",
                                         name=f"ps_{nt_off}_{gi}")
                        psv = ps[:, :ng * nt_sz].rearrange(
                            "p (g n) -> p g n", g=ng)
                        for h, mc in enumerate(mcs):
                            nc.tensor.matmul(
                                psv[:, h, :],
                                lhsT=fctx_sb[:, mc * 128:(mc + 1) * 128],
                                rhs=q_sb[:, nt_off:nt_off + nt_sz],
                                start=True, stop=True)
                        es = exps.tile([128, MGRP * 512], BF16, tag="es",
                                       name=f"es_{nt_off}_{gi}")
                        nc.scalar.activation(
                            out=es[:, :ng * nt_sz], in_=ps[:, :ng * nt_sz],
                            func=mybir.ActivationFunctionType.Exp)
                        cur = (es[:, :ng * nt_sz].rearrange(
                            "p (g n) -> p g n", g=ng), mcs)
                    else:
                        cur = None
                    if prev is not None:
                        es_p, mcs_p = prev
                        for h, mc in enumerate(mcs_p):
                            for ns in range(n_subs):
                                ns_sz = min(128, nt_sz - ns * 128)
                                nc.tensor.matmul(
                                    po[:ns_sz, ns >> 1, ns & 1, :],
                                    lhsT=es_p[:, h, ns * 128:ns * 128 + ns_sz],
                                    rhs=fmt_sb[:, mc, :],
                                    start=(mc == 0), stop=(mc == MCH - 1),
                                    skip_group_check=True)
                    prev = cur

                # epilogue: normalize, transpose, apply Wv*gamma, residual
                for ns in range(n_subs):
                    ns_sz = min(128, nt_sz - ns * 128)
                    recip = small.tile([128, 1], F32, tag="recip",
                                       name=f"rc_{nt_off}_{ns}")
                    nc.vector.reciprocal(
                        recip[:ns_sz], po[:ns_sz, ns >> 1, ns & 1, 128:129])
                    unorm = small.tile([128, 128], BF16, tag="unorm",
                                       name=f"un_{nt_off}_{ns}")
                    nc.vector.tensor_scalar_mul(
                        unorm[:ns_sz, :], po[:ns_sz, ns >> 1, ns & 1, 0:C],
                        recip[:ns_sz])
                    pt = psum_e.tile([128, 128], BF16, tag="pe",
                                     name=f"pt_{nt_off}_{ns}")
                    nc.tensor.transpose(pt[:, 0:ns_sz], unorm[:ns_sz, :],
                                        ident[:ns_sz, :ns_sz])
                    ut = small.tile([128, 128], BF16, tag="ut",
                                    name=f"ut_{nt_off}_{ns}")
                    nc.vector.tensor_copy(out=ut[:, :ns_sz], in_=pt[:, 0:ns_sz])
                    pf = psum_e.tile([128, 128], F32, tag="pe",
                                     name=f"pf_{nt_off}_{ns}")
                    nc.tensor.matmul(pf[:, :ns_sz], lhsT=wvg_sb[:],
                                     rhs=ut[:, :ns_sz], start=True, stop=True)
                    outt = small.tile([128, 128], F32, tag="outt",
                                      name=f"ot_{nt_off}_{ns}")
                    nc.vector.tensor_add(
                        out=outt[:, :ns_sz], in0=pf[:, :ns_sz],
                        in1=fres_sb[:, nt_off + ns * 128:
                                    nt_off + ns * 128 + ns_sz])
                    nc.sync.dma_start(
                        out[:, nt_off + ns * 128:nt_off + ns * 128 + ns_sz],
                        outt[:, :ns_sz])
    nc.compile()
    return nc


def kernel(feat_ctx, feat_mo, w_qk, w_v, gamma, itr=0, **_unused):
    feat_ctx = np.asarray(feat_ctx, dtype=np.float32).reshape(B, C, HW)
    feat_mo = np.asarray(feat_mo, dtype=np.float32).reshape(B, C, HW)
    w_qk = np.asarray(w_qk, dtype=np.float32)
    w_v = np.asarray(w_v, dtype=np.float32)
    gamma_v = float(np.asarray(gamma).reshape(-1)[0])

    bf = ml_dtypes.bfloat16
    w_q = w_qk[:C]
    w_k = w_qk[C:]
    # S = X.T (scale Wk.T Wq) X ; the Q'-projection matmul wants M.T as lhsT
    mqT = np.ascontiguousarray(SCALE * (w_q.T @ w_k)).astype(bf)
    wvgT = np.ascontiguousarray((gamma_v * w_v).T).astype(bf)

    fctx_bf = feat_ctx.astype(bf)

    if "nc" not in _CACHE:
        _CACHE["nc"] = _build()
    nc = _CACHE["nc"]

    ones_col = np.ones((C, MCH, 1), dtype=bf)
    in_maps = []
    for core in range(NCORES):
        b = core // CORES_PER_B
        s = (core % CORES_PER_B) * NSLC
        # Rotate the hw axis so this core's query slice is columns [0, NSLC).
        # K and the softmax sum over m are invariant to the rotation; V uses
        # the SAME rotation so the (k_m, v_m) pairing is intact.
        perm_ctx = np.ascontiguousarray(np.roll(fctx_bf[b], -s, axis=1))
        perm_mo = np.roll(feat_mo[b], -s, axis=1)
        # fmt[m_local, mc, c] = perm_mo[c, mc*128 + m_local]  (+ ones column)
        fmt = perm_mo.T.reshape(MCH, 128, C).transpose(1, 0, 2).astype(bf)
        fmt = np.ascontiguousarray(np.concatenate([fmt, ones_col], axis=2))
        in_maps.append({
            "fctx": perm_ctx,
            "fmt": fmt,
            "fres": np.ascontiguousarray(feat_mo[b][:, s:s + NSLC]),
            "mq": mqT, "wvg": wvgT,
        })

    trace = bool(int(os.environ.get("KERNEL_TRACE", "0")))
    res = run_bass_kernel_spmd(nc, in_maps, core_ids=list(range(NCORES)),
                               trace=trace)
    kernel.last_exec_time_ns = res.exec_time_ns

    out = np.empty((B, C, HW), dtype=np.float32)
    for core in range(NCORES):
        b = core // CORES_PER_B
        s = (core % CORES_PER_B) * NSLC
        out[b][:, s:s + NSLC] = res.results[core]["out"]
    return out.reshape(B, C, H, W)


# revision 14
# speedup vs baseline: 1.3581x; 1.0603x over previous
"""Trainium2 Bass kernel for nn_Aggregator (context attention aggregator).

Reference computation (per batch b, with c=128, hw=6400):
  q    = scale * (Wq @ X);  k = Wk @ X          # X = feat_ctx [128, hw]
  attn = softmax_over_m(k.T @ q)                # [m=hw, n=hw]
  out  = feat_mo + gamma * ((Wv @ feat_mo) @ attn)

Algebraic folds (host-side, exact f32):
  S = X.T @ M @ X  with  M = scale * Wk.T @ Wq  -> no K projection on device
  gamma * (Wv @ Fm) @ attn = Wvg @ (Fm @ attn)  -> Wv applied AFTER attention
    (Wvg = gamma * Wv); the host passes Fm pre-transposed to [m, c] layout
    with a ones column appended, so the same PSUM accumulation produces the
    softmax denominator for free.

Sharding: 8 cores, data-parallel over batch (4 cores/batch); each core owns
1600 query columns (the host rotates the hw axis per core so its slice is
always columns [0,1600) -- softmax over m is permutation invariant as long as
K and V use the same permutation).  Flash-style: the hw x hw attention matrix
never leaves PSUM/SBUF tiles.
"""

import os
import sys
import types

import numpy as np
import ml_dtypes

import concourse.bass as bass
import concourse.tile as tile
from concourse import bacc, mybir
from concourse.bass_utils import run_bass_kernel_spmd

# ---------------------------------------------------------------------------
# Environment fixes (self-contained on purpose: the grading harness imports
# only this file).
# ---------------------------------------------------------------------------


def _install_axon_profile_hook():
    """The image's `antenv` stub lacks `axon_hooks`; run_bass_kernel_spmd
    imports it when trace=True under axon.  Register a functional stand-in."""
    if "antenv.axon_hooks" in sys.modules:
        return
    mod = types.ModuleType("antenv.axon_hooks")
    _hook = [None]
    mod.set_axon_ntff_profile_hook = lambda h: _hook.__setitem__(0, h)
    mod.get_axon_ntff_profile_hook = lambda: _hook[0]
    sys.modules["antenv.axon_hooks"] = mod
    try:
        import antenv

        antenv.axon_hooks = mod
    except Exception:
        pass
    try:
        from trn_agent_boot.trn_boot import _ntff_profile_via_ctypes

        mod.set_axon_ntff_profile_hook(
            _ntff_profile_via_ctypes("/opt/axon/libaxon_pjrt.so")
        )
    except Exception:
        pass


def _install_tile_drain_patch():
    """walrus in this toolchain rejects >1 sync-wait on one CTRL instruction
    ("Too many sync wait commands").  TileContext's final drain carries one
    wait per live semaphore; split them onto individual SP nops."""
    if getattr(tile.TileContext, "_drain_patch_installed", False):
        return
    from concourse.vector_clock import ScopedClock

    def _patched(self, tick_clock, wait_clock):
        nc = self.nc
        probe = nc.sync.nop()
        wait_clock.add_sem_waits(
            probe.ins, ScopedClock({None: tick_clock.global_clock})
        )
        si = probe.ins.sync_info
        waits = list(si.on_wait) if si and si.on_wait else []
        if len(waits) > 1:
            si.on_wait = waits[:1]
            for w in waits[1:]:
                nw = nc.sync.nop()
                nsi = nw.ins.sync_info
                if nsi is None:
                    nw.ins.sync_info = mybir.SyncInfo(on_wait=[w], on_update=[])
                else:
                    nsi.on_wait = [w]
        nc.sync.drain()
        nc.all_engine_barrier()
        assert self.sems is not None
        popped = nc._tile_sem_poison_stack.pop()
        assert popped is self._sem_poison
        nc.clear_and_free_semaphores(list(self.sems.allocated().values()))
        nc.all_engine_barrier()

    tile.TileContext._drain_and_barrier = _patched
    tile.TileContext._drain_patch_installed = True


_install_axon_profile_hook()
_install_tile_drain_patch()

# ---------------------------------------------------------------------------
# Problem constants (hardcoded per spec)
# ---------------------------------------------------------------------------
B = 2          # batch
C = 128        # channels
H = W = 80
HW = H * W     # 6400
NCORES = 8
CORES_PER_B = NCORES // B      # 4
NSLC = HW // CORES_PER_B       # 1600 query columns per core
SCALE = C ** -0.5

MCH = HW // 128                # 50 m-chunks of 128
N_TILES = [(0, 512), (512, 512), (1024, 512), (1536, 64)]
# feat_ctx arrives as four separate SBUF tiles so early matmuls don't wait on
# the whole 1.6MB load (Tile deps are per-tile).  13/13/13/11 m-chunks.
FCTX_SPLIT = [(0, 1664), (1664, 1664), (3328, 1664), (4992, 1408)]
FMT_SPLIT = 5                  # fmt in 5 tiles of 10 m-chunks

F32 = mybir.dt.float32
BF16 = mybir.dt.bfloat16

_CACHE = {}


def _build():
    nc = bacc.Bacc("TRN2", target_bir_lowering=False, debug=False,
                   num_devices=NCORES)

    fctx = nc.dram_tensor("fctx", [C, HW], BF16, kind="ExternalInput").ap()
    fmt = nc.dram_tensor("fmt", [C, MCH, 129], BF16, kind="ExternalInput").ap()
    fres = nc.dram_tensor("fres", [C, NSLC], F32, kind="ExternalInput").ap()
    mq = nc.dram_tensor("mq", [C, C], BF16, kind="ExternalInput").ap()
    wvg = nc.dram_tensor("wvg", [C, C], BF16, kind="ExternalInput").ap()
    out = nc.dram_tensor("out", [C, NSLC], F32, kind="ExternalOutput").ap()

    with tile.TileContext(nc) as tc:
        with (
            tc.tile_pool(name="weights", bufs=1) as wpool,
            tc.tile_pool(name="io", bufs=1) as io,
            tc.tile_pool(name="exps", bufs=3) as exps,
            tc.tile_pool(name="small", bufs=3) as small,
            tc.tile_pool(name="psum_s", bufs=2, space="PSUM") as psum_s,
            tc.tile_pool(name="psum_o", bufs=2, space="PSUM") as psum_o,
        ):
            # ---- load inputs: weights + first fctx tile first --------------
            mq_sb = wpool.tile([C, C], BF16)
            nc.sync.dma_start(mq_sb[:], mq[:])

            fctx_sb = []
            for i, (off, sz) in enumerate(FCTX_SPLIT):
                t = io.tile([C, sz], BF16, name=f"fctx{i}")
                fctx_sb.append(t)
            nc.sync.dma_start(fctx_sb[0][:], fctx[:, 0:1664])

            fmt_sb = []
            for j in range(FMT_SPLIT):
                t = io.tile([C, 10, 129], BF16, name=f"fmt{j}")
                fmt_sb.append(t)
            nc.sync.dma_start(fmt_sb[0][:], fmt[:, 0:10, :])
            wvg_sb = wpool.tile([C, C], BF16)
            nc.sync.dma_start(wvg_sb[:], wvg[:])
            for i, (off, sz) in list(enumerate(FCTX_SPLIT))[1:]:
                nc.sync.dma_start(fctx_sb[i][:], fctx[:, off:off + sz])
            for j in range(1, FMT_SPLIT):
                nc.sync.dma_start(fmt_sb[j][:], fmt[:, j * 10:(j + 1) * 10, :])
            fres_sb = io.tile([C, NSLC], F32)
            nc.sync.dma_start(fres_sb[:], fres[:])

            ident = wpool.tile([C, C], BF16)
            from concourse.masks import make_identity
            make_identity(nc, ident)

            def fctx_slice(lo, hi):
                for (off, sz), t in zip(FCTX_SPLIT, fctx_sb):
                    if off <= lo and hi <= off + sz:
                        return t[:, lo - off:hi - off]
                raise AssertionError((lo, hi))

            def fmt_slice(mc):
                return fmt_sb[mc // 10][:, mc % 10, :]

            # ---- Q' projection: q'[i, n] = sum_j M[i, j] X[j, n] -----------
            q_sb = io.tile([C, NSLC], BF16)
            for nt_off, nt_sz in N_TILES:
                ps = psum_s.tile([128, 1024], F32, tag="ps", name="psq")
                nc.tensor.matmul(ps[:, :nt_sz], lhsT=mq_sb[:],
                                 rhs=fctx_slice(nt_off, nt_off + nt_sz),
                                 start=True, stop=True)
                nc.vector.tensor_copy(out=q_sb[:, nt_off:nt_off + nt_sz],
                                      in_=ps[:, :nt_sz])

            # ---- attention -------------------------------------------------
            epilogue_q = []   # deferred per-ns epilogues (closures)

            def emit_epilogue(po, nt_off, nt_sz, ns):
                ns_sz = min(128, nt_sz - ns * 128)
                sfx = f"{nt_off}_{ns}"
                recip = small.tile([128, 1], F32, tag="recip", name=f"rc{sfx}")
                nc.vector.reciprocal(
                    recip[:ns_sz], po[:ns_sz, ns >> 1, ns & 1, 128:129])
                unorm = small.tile([128, 128], BF16, tag="unorm",
                                   name=f"un{sfx}")
                nc.vector.tensor_scalar_mul(
                    unorm[:ns_sz, :], po[:ns_sz, ns >> 1, ns & 1, 0:C],
                    recip[:ns_sz])
                pt = psum_s.tile([128, 128], BF16, tag="ps", name=f"pt{sfx}")
                nc.tensor.transpose(pt[:, 0:ns_sz], unorm[:ns_sz, :],
                                    ident[:ns_sz, :ns_sz])
                ut = small.tile([128, 128], BF16, tag="ut", name=f"ut{sfx}")
                nc.vector.tensor_copy(out=ut[:, :ns_sz], in_=pt[:, 0:ns_sz])
                pf = psum_s.tile([128, 128], F32, tag="ps", name=f"pf{sfx}")
                nc.tensor.matmul(pf[:, :ns_sz], lhsT=wvg_sb[:],
                                 rhs=ut[:, :ns_sz], start=True, stop=True)
                outt = small.tile([128, 128], F32, tag="outt", name=f"ot{sfx}")
                nc.vector.tensor_add(
                    out=outt[:, :ns_sz], in0=pf[:, :ns_sz],
                    in1=fres_sb[:, nt_off + ns * 128:
                                nt_off + ns * 128 + ns_sz])
                nc.sync.dma_start(
                    out[:, nt_off + ns * 128:nt_off + ns * 128 + ns_sz],
                    outt[:, :ns_sz])

            for nt_off, nt_sz in N_TILES:
                n_subs = (nt_sz + 127) // 128
                mgrp = 2 if nt_sz > 256 else 16
                # accumulators: po[:, ns >> 1, ns & 1, :] is one [*, 129]
                # block; pairs stay inside one 512-f32 PSUM bank.
                po = psum_o.tile([128, (n_subs + 1) // 2, 2, 129], F32,
                                 tag="po", name=f"po_{nt_off}")
                groups = [list(range(g, min(g + mgrp, MCH)))
                          for g in range(0, MCH, mgrp)]
                prev = None  # (expS 3d view, chunk list) pending AV
                for gi in range(len(groups) + 1):
                    # drain one deferred epilogue per group slot
                    if epilogue_q and 2 <= gi:
                        epilogue_q.pop(0)()
                    if gi < len(groups):
                        mcs = groups[gi]
                        ng = len(mcs)
                        ps = psum_s.tile([128, 1024], F32, tag="ps",
                                         name=f"ps_{nt_off}_{gi}")
                        psv = ps[:, :ng * nt_sz].rearrange(
                            "p (g n) -> p g n", g=ng)
                        for h, mc in enumerate(mcs):
                            nc.tensor.matmul(
                                psv[:, h, :],
                                lhsT=fctx_slice(mc * 128, (mc + 1) * 128),
                                rhs=q_sb[:, nt_off:nt_off + nt_sz],
                                start=True, stop=True)
                        es = exps.tile([128, 1024], BF16, tag="es",
                                       name=f"es_{nt_off}_{gi}")
                        nc.scalar.activation(
                            out=es[:, :ng * nt_sz], in_=ps[:, :ng * nt_sz],
                            func=mybir.ActivationFunctionType.Exp)
                        cur = (es[:, :ng * nt_sz].rearrange(
                            "p (g n) -> p g n", g=ng), mcs)
                    else:
                        cur = None
                    if prev is not None:
                        es_p, mcs_p = prev
                        for h, mc in enumerate(mcs_p):
                            for ns in range(n_subs):
                                ns_sz = min(128, nt_sz - ns * 128)
                                nc.tensor.matmul(
                                    po[:ns_sz, ns >> 1, ns & 1, :],
                                    lhsT=es_p[:, h, ns * 128:ns * 128 + ns_sz],
                                    rhs=fmt_slice(mc),
                                    start=(mc == 0), stop=(mc == MCH - 1),
                                    skip_group_check=True)
                    prev = cur

                for ns in range(n_subs):
                    epilogue_q.append(
                        lambda po=po, nt_off=nt_off, nt_sz=nt_sz, ns=ns:
                        emit_epilogue(po, nt_off, nt_sz, ns))

            for fn in epilogue_q:
                fn()
    nc.compile()
    return nc


def kernel(feat_ctx, feat_mo, w_qk, w_v, gamma, itr=0, **_unused):
    feat_ctx = np.asarray(feat_ctx, dtype=np.float32).reshape(B, C, HW)
    feat_mo = np.asarray(feat_mo, dtype=np.float32).reshape(B, C, HW)
    w_qk = np.asarray(w_qk, dtype=np.float32)
    w_v = np.asarray(w_v, dtype=np.float32)
    gamma_v = float(np.asarray(gamma).reshape(-1)[0])

    bf = ml_dtypes.bfloat16
    w_q = w_qk[:C]
    w_k = w_qk[C:]
    # S = X.T (scale Wk.T Wq) X ; the Q'-projection matmul wants M.T as lhsT
    mqT = np.ascontiguousarray(SCALE * (w_q.T @ w_k)).astype(bf)
    wvgT = np.ascontiguousarray((gamma_v * w_v).T).astype(bf)

    fctx_bf = feat_ctx.astype(bf)

    if "nc" not in _CACHE:
        _CACHE["nc"] = _build()
    nc = _CACHE["nc"]

    ones_col = np.ones((C, MCH, 1), dtype=bf)
    in_maps = []
    for core in range(NCORES):
        b = core // CORES_PER_B
        s = (core % CORES_PER_B) * NSLC
        # Rotate the hw axis so this core's query slice is columns [0, NSLC).
        # K and the softmax sum over m are invariant to the rotation; V uses
        # the SAME rotation so the (k_m, v_m) pairing is intact.
        perm_ctx = np.ascontiguousarray(np.roll(fctx_bf[b], -s, axis=1))
        perm_mo = np.roll(feat_mo[b], -s, axis=1)
        # fmt[m_local, mc, c] = perm_mo[c, mc*128 + m_local]  (+ ones column)
        fmt = perm_mo.T.reshape(MCH, 128, C).transpose(1, 0, 2).astype(bf)
        fmt = np.ascontiguousarray(np.concatenate([fmt, ones_col], axis=2))
        in_maps.append({
            "fctx": perm_ctx,
            "fmt": fmt,
            "fres": np.ascontiguousarray(feat_mo[b][:, s:s + NSLC]),
            "mq": mqT, "wvg": wvgT,
        })

    trace = bool(int(os.environ.get("KERNEL_TRACE", "0")))
    res = run_bass_kernel_spmd(nc, in_maps, core_ids=list(range(NCORES)),
                               trace=trace)
    kernel.last_exec_time_ns = res.exec_time_ns

    out = np.empty((B, C, HW), dtype=np.float32)
    for core in range(NCORES):
        b = core // CORES_PER_B
        s = (core % CORES_PER_B) * NSLC
        out[b][:, s:s + NSLC] = res.results[core]["out"]
    return out.reshape(B, C, H, W)


# revision 18
# speedup vs baseline: 1.4333x; 1.0554x over previous
"""Trainium2 Bass kernel for nn_Aggregator (context attention aggregator).

Reference computation (per batch b, with c=128, hw=6400):
  q    = scale * (Wq @ X);  k = Wk @ X          # X = feat_ctx [128, hw]
  attn = softmax_over_m(k.T @ q)                # [m=hw, n=hw]
  out  = feat_mo + gamma * ((Wv @ feat_mo) @ attn)

Algebraic folds (host-side, exact f32):
  S = X.T @ M @ X  with  M = scale * Wk.T @ Wq  -> no K projection on device
  gamma * (Wv @ Fm) @ attn = Wvg @ (Fm @ attn)  -> Wv applied AFTER attention
    (Wvg = gamma * Wv); the host passes Fm pre-transposed to [m, c] layout
    with a ones column appended, so the same PSUM accumulation produces the
    softmax denominator for free.

Sharding: 8 cores, data-parallel over batch (4 cores/batch); each core owns
1600 query columns (the host rotates the hw axis per core so its slice is
always columns [0,1600) -- softmax over m is permutation invariant as long as
K and V use the same permutation).  Flash-style: the hw x hw attention matrix
never leaves PSUM/SBUF tiles.
"""

import os
import sys
import types

import numpy as np
import ml_dtypes

import concourse.bass as bass
import concourse.tile as tile
from concourse import bacc, mybir
from concourse.bass_utils import run_bass_kernel_spmd

# ---------------------------------------------------------------------------
# Environment fixes (self-contained on purpose: the grading harness imports
# only this file).
# ---------------------------------------------------------------------------


def _install_axon_profile_hook():
    """The image's `antenv` stub lacks `axon_hooks`; run_bass_kernel_spmd
    imports it when trace=True under axon.  Register a functional stand-in."""
    if "antenv.axon_hooks" in sys.modules:
        return
    mod = types.ModuleType("antenv.axon_hooks")
    _hook = [None]
    mod.set_axon_ntff_profile_hook = lambda h: _hook.__setitem__(0, h)
    mod.get_axon_ntff_profile_hook = lambda: _hook[0]
    sys.modules["antenv.axon_hooks"] = mod
    try:
        import antenv

        antenv.axon_hooks = mod
    except Exception:
        pass
    try:
        from trn_agent_boot.trn_boot import _ntff_profile_via_ctypes

        mod.set_axon_ntff_profile_hook(
            _ntff_profile_via_ctypes("/opt/axon/libaxon_pjrt.so")
        )
    except Exception:
        pass


def _install_tile_drain_patch():
    """walrus in this toolchain rejects >1 sync-wait on one CTRL instruction
    ("Too many sync wait commands").  TileContext's final drain carries one
    wait per live semaphore; split them onto individual SP nops."""
    if getattr(tile.TileContext, "_drain_patch_installed", False):
        return
    from concourse.vector_clock import ScopedClock

    def _patched(self, tick_clock, wait_clock):
        nc = self.nc
        probe = nc.sync.nop()
        wait_clock.add_sem_waits(
            probe.ins, ScopedClock({None: tick_clock.global_clock})
        )
        si = probe.ins.sync_info
        waits = list(si.on_wait) if si and si.on_wait else []
        if len(waits) > 1:
            si.on_wait = waits[:1]
            for w in waits[1:]:
                nw = nc.sync.nop()
                nsi = nw.ins.sync_info
                if nsi is None:
                    nw.ins.sync_info = mybir.SyncInfo(on_wait=[w], on_update=[])
                else:
                    nsi.on_wait = [w]
        nc.sync.drain()
        nc.all_engine_barrier()
        assert self.sems is not None
        popped = nc._tile_sem_poison_stack.pop()
        assert popped is self._sem_poison
        nc.clear_and_free_semaphores(list(self.sems.allocated().values()))
        nc.all_engine_barrier()

    tile.TileContext._drain_and_barrier = _patched
    tile.TileContext._drain_patch_installed = True


_install_axon_profile_hook()
_install_tile_drain_patch()

# ---------------------------------------------------------------------------
# Problem constants (hardcoded per spec)
# ---------------------------------------------------------------------------
B = 2          # batch
C = 128        # channels
H = W = 80
HW = H * W     # 6400
NCORES = 8
CORES_PER_B = NCORES // B      # 4
NSLC = HW // CORES_PER_B       # 1600 query columns per core
SCALE = C ** -0.5

MCH = HW // 128                # 50 m-chunks of 128
N_TILES = [(0, 512), (512, 512), (1024, 512), (1536, 64)]
# feat_ctx arrives as separate SBUF tiles so early matmuls don't wait on the
# whole 1.6MB load (Tile deps are per-tile).  4/9/13/13/11 m-chunks.
FCTX_SPLIT = [(0, 512), (512, 1152), (1664, 1664), (3328, 1664), (4992, 1408)]
FMT_SPLIT = 5                  # fmt in 5 tiles of 10 m-chunks

F32 = mybir.dt.float32
BF16 = mybir.dt.bfloat16

_CACHE = {}


def _build():
    nc = bacc.Bacc("TRN2", target_bir_lowering=False, debug=False,
                   num_devices=NCORES)

    fctx = nc.dram_tensor("fctx", [C, HW], BF16, kind="ExternalInput").ap()
    fmt = nc.dram_tensor("fmt", [C, MCH, 129], BF16, kind="ExternalInput").ap()
    fres = nc.dram_tensor("fres", [C, NSLC], F32, kind="ExternalInput").ap()
    mq = nc.dram_tensor("mq", [C, C], BF16, kind="ExternalInput").ap()
    wvg = nc.dram_tensor("wvg", [C, C], BF16, kind="ExternalInput").ap()
    out = nc.dram_tensor("out", [C, NSLC], F32, kind="ExternalOutput").ap()

    with tile.TileContext(nc) as tc:
        with (
            tc.tile_pool(name="weights", bufs=1) as wpool,
            tc.tile_pool(name="io", bufs=1) as io,
            tc.tile_pool(name="exps", bufs=4) as exps,
            tc.tile_pool(name="small", bufs=3) as small,
            tc.tile_pool(name="psum_s", bufs=2, space="PSUM") as psum_s,
            tc.tile_pool(name="psum_o", bufs=2, space="PSUM") as psum_o,
        ):
            # ---- load inputs: weights + first fctx tile first --------------
            mq_sb = wpool.tile([C, C], BF16)
            nc.sync.dma_start(mq_sb[:], mq[:])

            fctx_sb = []
            for i, (off, sz) in enumerate(FCTX_SPLIT):
                t = io.tile([C, sz], BF16, name=f"fctx{i}")
                fctx_sb.append(t)
                nc.sync.dma_start(t[:], fctx[:, off:off + sz])

            # fmt loads issue from the gpsimd sequencer in parallel with the
            # sync sequencer's fctx issues (each dma_start costs ~0.6us of
            # issue time on its sequencer).
            fmt_sb = []
            for j in range(FMT_SPLIT):
                t = io.tile([C, 10, 129], BF16, name=f"fmt{j}")
                fmt_sb.append(t)
                nc.gpsimd.dma_start(t[:], fmt[:, j * 10:(j + 1) * 10, :])
            wvg_sb = wpool.tile([C, C], BF16)
            nc.sync.dma_start(wvg_sb[:], wvg[:])
            fres_sb = io.tile([C, NSLC], F32)
            nc.sync.dma_start(fres_sb[:], fres[:])

            ident = wpool.tile([C, C], BF16)
            from concourse.masks import make_identity
            make_identity(nc, ident)

            def fctx_slice(lo, hi):
                for (off, sz), t in zip(FCTX_SPLIT, fctx_sb):
                    if off <= lo and hi <= off + sz:
                        return t[:, lo - off:hi - off]
                raise AssertionError((lo, hi))

            def fmt_slice(mc):
                return fmt_sb[mc // 10][:, mc % 10, :]

            # ---- Q' projection: q'[i, n] = sum_j M[i, j] X[j, n] -----------
            q_sb = io.tile([C, NSLC], BF16)
            for nt_off, nt_sz in N_TILES:
                ps = psum_s.tile([128, 1024], F32, tag="ps", name="psq")
                nc.tensor.matmul(ps[:, :nt_sz], lhsT=mq_sb[:],
                                 rhs=fctx_slice(nt_off, nt_off + nt_sz),
                                 start=True, stop=True)
                nc.vector.tensor_copy(out=q_sb[:, nt_off:nt_off + nt_sz],
                                      in_=ps[:, :nt_sz])

            # ---- attention -------------------------------------------------
            epilogue_q = []   # deferred per-ns epilogues (closures)

            def emit_epilogue(po, nt_off, nt_sz, ns):
                ns_sz = min(128, nt_sz - ns * 128)
                sfx = f"{nt_off}_{ns}"
                recip = small.tile([128, 1], F32, tag="recip", name=f"rc{sfx}")
                nc.vector.reciprocal(
                    recip[:ns_sz], po[:ns_sz, ns >> 1, ns & 1, 128:129])
                unorm = small.tile([128, 128], BF16, tag="unorm",
                                   name=f"un{sfx}")
                nc.vector.tensor_scalar_mul(
                    unorm[:ns_sz, :], po[:ns_sz, ns >> 1, ns & 1, 0:C],
                    recip[:ns_sz])
                pt = psum_s.tile([128, 128], BF16, tag="ps", name=f"pt{sfx}")
                nc.tensor.transpose(pt[:, 0:ns_sz], unorm[:ns_sz, :],
                                    ident[:ns_sz, :ns_sz])
                ut = small.tile([128, 128], BF16, tag="ut", name=f"ut{sfx}")
                nc.vector.tensor_copy(out=ut[:, :ns_sz], in_=pt[:, 0:ns_sz])
                pf = psum_s.tile([128, 128], F32, tag="ps", name=f"pf{sfx}")
                nc.tensor.matmul(pf[:, :ns_sz], lhsT=wvg_sb[:],
                                 rhs=ut[:, :ns_sz], start=True, stop=True)
                outt = small.tile([128, 128], F32, tag="outt", name=f"ot{sfx}")
                nc.vector.tensor_add(
                    out=outt[:, :ns_sz], in0=pf[:, :ns_sz],
                    in1=fres_sb[:, nt_off + ns * 128:
                                nt_off + ns * 128 + ns_sz])
                nc.sync.dma_start(
                    out[:, nt_off + ns * 128:nt_off + ns * 128 + ns_sz],
                    outt[:, :ns_sz])

            # Flat software pipeline over every (n-tile, m-group): at step i
            # emit S+exp for group i and the AV matmuls for group i-2, so the
            # PE never waits on an exp that is still in flight.
            steps = []
            for nt_off, nt_sz in N_TILES:
                mgrp = 2 if nt_sz > 256 else 16
                for g in range(0, MCH, mgrp):
                    steps.append((nt_off, nt_sz,
                                  list(range(g, min(g + mgrp, MCH)))))
            po_map = {}
            pend = []

            def emit_av(item):
                es_p, mcs_p, po, nt_off, nt_sz = item
                n_subs = (nt_sz + 127) // 128
                for h, mc in enumerate(mcs_p):
                    for ns in range(n_subs):
                        ns_sz = min(128, nt_sz - ns * 128)
                        nc.tensor.matmul(
                            po[:ns_sz, ns >> 1, ns & 1, :],
                            lhsT=es_p[:, h, ns * 128:ns * 128 + ns_sz],
                            rhs=fmt_slice(mc),
                            start=(mc == 0), stop=(mc == MCH - 1),
                            skip_group_check=True)
                if mcs_p[-1] == MCH - 1:
                    for ns in range(n_subs):
                        epilogue_q.append(
                            lambda po=po, nt_off=nt_off, nt_sz=nt_sz, ns=ns:
                            emit_epilogue(po, nt_off, nt_sz, ns))

            for si in range(len(steps) + 2):
                if si < len(steps):
                    nt_off, nt_sz, mcs = steps[si]
                    n_subs = (nt_sz + 127) // 128
                    if nt_off not in po_map:
                        # po[:, ns >> 1, ns & 1, :] is one [*, 129] block;
                        # pairs stay inside one 512-f32 PSUM bank.
                        po_map[nt_off] = psum_o.tile(
                            [128, (n_subs + 1) // 2, 2, 129], F32,
                            tag="po", name=f"po_{nt_off}")
                    ng = len(mcs)
                    ps = psum_s.tile([128, 1024], F32, tag="ps",
                                     name=f"ps_{nt_off}_{mcs[0]}")
                    psv = ps[:, :ng * nt_sz].rearrange(
                        "p (g n) -> p g n", g=ng)
                    for h, mc in enumerate(mcs):
                        nc.tensor.matmul(
                            psv[:, h, :],
                            lhsT=fctx_slice(mc * 128, (mc + 1) * 128),
                            rhs=q_sb[:, nt_off:nt_off + nt_sz],
                            start=True, stop=True)
                    es = exps.tile([128, 1024], BF16, tag="es",
                                   name=f"es_{nt_off}_{mcs[0]}")
                    nc.scalar.activation(
                        out=es[:, :ng * nt_sz], in_=ps[:, :ng * nt_sz],
                        func=mybir.ActivationFunctionType.Exp)
                    pend.append((es[:, :ng * nt_sz].rearrange(
                        "p (g n) -> p g n", g=ng), mcs,
                        po_map[nt_off], nt_off, nt_sz))
                if len(pend) > 2 or (si >= len(steps) and pend):
                    emit_av(pend.pop(0))
                if epilogue_q:
                    epilogue_q.pop(0)()

            while epilogue_q:
                epilogue_q.pop(0)()
    nc.compile()
    return nc


def kernel(feat_ctx, feat_mo, w_qk, w_v, gamma, itr=0, **_unused):
    feat_ctx = np.asarray(feat_ctx, dtype=np.float32).reshape(B, C, HW)
    feat_mo = np.asarray(feat_mo, dtype=np.float32).reshape(B, C, HW)
    w_qk = np.asarray(w_qk, dtype=np.float32)
    w_v = np.asarray(w_v, dtype=np.float32)
    gamma_v = float(np.asarray(gamma).reshape(-1)[0])

    bf = ml_dtypes.bfloat16
    w_q = w_qk[:C]
    w_k = w_qk[C:]
    # S = X.T (scale Wk.T Wq) X ; the Q'-projection matmul wants M.T as lhsT
    mqT = np.ascontiguousarray(SCALE * (w_q.T @ w_k)).astype(bf)
    wvgT = np.ascontiguousarray((gamma_v * w_v).T).astype(bf)

    fctx_bf = feat_ctx.astype(bf)

    if "nc" not in _CACHE:
        _CACHE["nc"] = _build()
    nc = _CACHE["nc"]

    ones_col = np.ones((C, MCH, 1), dtype=bf)
    in_maps = []
    for core in range(NCORES):
        b = core // CORES_PER_B
        s = (core % CORES_PER_B) * NSLC
        # Rotate the hw axis so this core's query slice is columns [0, NSLC).
        # K and the softmax sum over m are invariant to the rotation; V uses
        # the SAME rotation so the (k_m, v_m) pairing is intact.
        perm_ctx = np.ascontiguousarray(np.roll(fctx_bf[b], -s, axis=1))
        perm_mo = np.roll(feat_mo[b], -s, axis=1)
        # fmt[m_local, mc, c] = perm_mo[c, mc*128 + m_local]  (+ ones column)
        fmt = perm_mo.T.reshape(MCH, 128, C).transpose(1, 0, 2).astype(bf)
        fmt = np.ascontiguousarray(np.concatenate([fmt, ones_col], axis=2))
        in_maps.append({
            "fctx": perm_ctx,
            "fmt": fmt,
            "fres": np.ascontiguousarray(feat_mo[b][:, s:s + NSLC]),
            "mq": mqT, "wvg": wvgT,
        })

    trace = bool(int(os.environ.get("KERNEL_TRACE", "0")))
    res = run_bass_kernel_spmd(nc, in_maps, core_ids=list(range(NCORES)),
                               trace=trace)
    kernel.last_exec_time_ns = res.exec_time_ns

    out = np.empty((B, C, HW), dtype=np.float32)
    for core in range(NCORES):
        b = core // CORES_PER_B
        s = (core % CORES_PER_B) * NSLC
        out[b][:, s:s + NSLC] = res.results[core]["out"]
    return out.reshape(B, C, H, W)
